# revision 1
# baseline (speedup 1.0000x reference)
"""DeepAR (2-layer LSTM encoder/decoder + gaussian heads) on 8 Trainium2 cores.

Strategy: pure data-parallel over batch B=1024 -> 128 rows/core (= SBUF
partition count). Everything on-chip per core:
  - batch-major layout [128 batch partitions, features] for states/elementwise
  - matmuls: out[b, gates] = lhsT.T @ rhs with lhsT = transposed activations
    (x^T / h^T chunks, [K<=128, 128]) stationary and rhs = pre-transposed
    weight chunks [K, 2048] moving, fp32r (full fp32 storage, fast PE mode),
    N-chunks of 512 into PSUM banks.
  - h^T produced each step via PE transpose + DVE copy-back (fp32->fp32r).
  - LSTM cell elementwise on DVE (bias add from PSUM, products) + ScalarE
    (sigmoid/tanh), biases pre-replicated across partitions host-side.

Per step the PE emission order interleaves the two layers so each layer's
elementwise latency hides under the other layer's independent matmul half:
  L0-h MMs | tpose h1(prev) | [dec: heads] | L0-x MMs | L0 el |
  L1-h MMs | tpose h0 | L1-x MMs | L1 el
"""

import numpy as np

import concourse.bass as bass
import concourse.mybir as mybir
import concourse.tile as tile
from concourse.bacc import Bacc
from concourse.bass_utils import run_bass_kernel_spmd

f32 = mybir.dt.float32
f32r = mybir.dt.float32r
AF = mybir.ActivationFunctionType
OP = mybir.AluOpType

B, T, D, H, K_OUT, TAU = 1024, 168, 32, 512, 8, 24
NCORES = 8
BC = B // NCORES          # 128 batch rows per core
G = 4 * H                 # 2048 gate width
NB = G // 512             # 4 psum n-chunks
HK = H // 128             # 4 k-chunks per hidden input


def build_nc(t_enc=T, t_dec=TAU):
    nc = Bacc()

    x_d = nc.dram_tensor("x", [BC, t_enc, D], f32, kind="ExternalInput")
    w_d = {}
    w_d["e0"] = nc.dram_tensor("w_e0", [1 + HK, 128, G], f32r, kind="ExternalInput")
    for nm in ("e1", "d0", "d1"):
        w_d[nm] = nc.dram_tensor(f"w_{nm}", [2 * HK, 128, G], f32r, kind="ExternalInput")
    b_d = {nm: nc.dram_tensor(f"b_{nm}", [BC, G], f32, kind="ExternalInput")
           for nm in ("e0", "e1", "d0", "d1")}
    wh_d = nc.dram_tensor("w_head", [HK, 128, 2 * K_OUT], f32r, kind="ExternalInput")
    bh_d = nc.dram_tensor("b_head", [BC, 2 * K_OUT], f32, kind="ExternalInput")
    id_d = nc.dram_tensor("ident", [128, 128], f32, kind="ExternalInput")
    mu_d = nc.dram_tensor("mu", [BC, t_dec, K_OUT], f32, kind="ExternalOutput")
    sg_d = nc.dram_tensor("sigma", [BC, t_dec, K_OUT], f32, kind="ExternalOutput")

    with tile.TileContext(nc) as tc:
        with (
            tc.tile_pool(name="consts", bufs=1) as consts,
            tc.tile_pool(name="wpool", bufs=16) as wpool,
            tc.tile_pool(name="bpool", bufs=2) as bpool,
            tc.tile_pool(name="xring", bufs=4) as xring,
            tc.tile_pool(name="tmps", bufs=12) as tmps,
            tc.tile_pool(name="gps", bufs=6, space="PSUM") as gps,
            tc.tile_pool(name="tps", bufs=2, space="PSUM") as tps,
        ):
            # ---------- startup loads ----------
            ident = consts.tile([128, 128], f32, tag="ident")
            nc.sync.dma_start(ident, id_d[:, :])

            bias = {}

            def load_bias(nm):
                bias[nm] = bpool.tile([BC, G], f32, tag="b", name=f"b_{nm}")
                nc.sync.dma_start(bias[nm], b_d[nm][:, :])

            load_bias("e0")
            load_bias("e1")

            w_head = consts.tile([128, HK, 2 * K_OUT], f32r, tag="w_head")
            nc.sync.dma_start(w_head, wh_d[:, :, :].rearrange("k p n -> p k n"))
            b_head = consts.tile([BC, 2 * K_OUT], f32, tag="b_head")
            nc.sync.dma_start(b_head, bh_d[:, :])

            def load_w(nm, nk):
                chunks = []
                for k in range(nk):
                    wt = wpool.tile([128, G], f32r, tag="w")
                    nc.sync.dma_start(wt, w_d[nm][k, :, :])
                    chunks.append(wt)
                return chunks

            w = {"e0": load_w("e0", 1 + HK), "e1": load_w("e1", 2 * HK)}

            # ---------- persistent state ----------
            c_st = {}
            hT = {}
            h_tmp = {}
            z0 = consts.tile([128, HK, 128], f32, tag="z0")
            nc.vector.memset(z0, 0.0)
            for l in (0, 1):
                c_st[l] = consts.tile([BC, H], f32, tag=f"c{l}", name=f"c{l}")
                nc.vector.memset(c_st[l], 0.0)
                hT[l] = consts.tile([128, HK, 128], f32r, tag=f"hT{l}", name=f"hT{l}")
                nc.vector.tensor_copy(hT[l], z0)
                h_tmp[l] = consts.tile([BC, H], f32, tag=f"h_tmp{l}", name=f"h_tmp{l}")

            mu_sb = consts.tile([BC, t_dec * K_OUT], f32, tag="mu_sb")
            zs_sb = consts.tile([BC, t_dec * K_OUT], f32, tag="zs_sb")
            sg_sb = consts.tile([BC, t_dec * K_OUT], f32, tag="sg_sb")

            # persistent xT ring; rows D..127 zeroed once (stale SBUF bytes
            # could be NaN patterns and NaN*0 would poison the gates)
            XRING = 4
            xT_ring = []
            for j in range(XRING):
                xt = consts.tile([128, 128], f32r, tag=f"xT{j}", name=f"xT{j}")
                nc.vector.tensor_copy(xt, z0[:, 0, :])
                xT_ring.append(xt)

            # ---------- helpers ----------
            def x_pipeline(t):
                """DMA x[:, t, :], funnel via DVE, transpose into the ring."""
                xs = xring.tile([BC, D], f32, tag="xs")
                nc.sync.dma_start(xs, x_d[:, t, :])
                xf = xring.tile([BC, D], f32, tag="xf")
                nc.vector.tensor_copy(xf, xs)
                xp = tps.tile([128, 128], f32, tag="tp")
                nc.tensor.transpose(xp[:D, :], xf, ident)
                nc.vector.tensor_copy(xT_ring[t % XRING][:D, :], xp[:D, :])

            import os as _os
            MM_ORDER = _os.environ.get("MM_ORDER", "n")

            def emit_mms(psums, w_chunks, lhsT_list, k_ids, start, stop):
                """Matmuls accumulating into psums[n]; n-outer frees PSUM
                banks early, k-outer reuses the stationary operand."""
                for n in range(NB):
                    if psums[n] is None:
                        psums[n] = gps.tile([BC, 512], f32, tag="g", name=f"g{n}")
                loop = ([(n, j) for n in range(NB) for j in range(len(k_ids))]
                        if MM_ORDER == "n" else
                        [(n, j) for j in range(len(k_ids)) for n in range(NB)])
                for n, j in loop:
                    nc.tensor.matmul(
                        psums[n],
                        lhsT_list[j],
                        w_chunks[k_ids[j]][:, n * 512:(n + 1) * 512],
                        start=start and j == 0,
                        stop=stop and j == len(k_ids) - 1,
                    )
                return psums

            def emit_tpose(l):
                """h_tmp[l] -> hT[l] (4 PE transposes + DVE copyback)."""
                for k in range(HK):
                    pt = tps.tile([128, 128], f32, tag="tp")
                    nc.tensor.transpose(pt, h_tmp[l][:, k * 128:(k + 1) * 128], ident)
                    nc.vector.tensor_copy(hT[l][:, k, :], pt)

            def emit_el(l, psums, b_t):
                """LSTM cell elementwise: gates in psums (i,f,g,o), updates
                c_st[l] in place and writes h_tmp[l]."""
                zb = []
                for gidx in range(4):
                    z = tmps.tile([BC, 512], f32, tag="e")
                    nc.vector.tensor_tensor(
                        z, psums[gidx], b_t[:, gidx * 512:(gidx + 1) * 512], OP.add)
                    zb.append(z)
                si = tmps.tile([BC, 512], f32, tag="e")
                nc.scalar.activation(si, zb[0], AF.Sigmoid)
                sf = tmps.tile([BC, 512], f32, tag="e")
                nc.scalar.activation(sf, zb[1], AF.Sigmoid)
                tg = tmps.tile([BC, 512], f32, tag="e")
                nc.scalar.activation(tg, zb[2], AF.Tanh)
                so = tmps.tile([BC, 512], f32, tag="e")
                nc.scalar.activation(so, zb[3], AF.Sigmoid)
                t2 = tmps.tile([BC, 512], f32, tag="e")
                nc.vector.tensor_tensor(t2, si, tg, OP.mult)
                nc.vector.tensor_tensor(c_st[l], c_st[l], sf, OP.mult)
                nc.vector.tensor_tensor(c_st[l], c_st[l], t2, OP.add)
                tc_ = tmps.tile([BC, 512], f32, tag="e")
                nc.scalar.activation(tc_, c_st[l], AF.Tanh)
                nc.vector.tensor_tensor(h_tmp[l], so, tc_, OP.mult)

            def emit_heads(ti):
                """mu/sigma for decoder output index ti from hT[1]."""
                hp = tps.tile([128, 128], f32, tag="tp")
                for k in range(HK):
                    nc.tensor.matmul(
                        hp[:, :2 * K_OUT], hT[1][:, k, :], w_head[:, k, :],
                        start=(k == 0), stop=(k == HK - 1))
                sl = slice(ti * K_OUT, (ti + 1) * K_OUT)
                nc.vector.tensor_tensor(
                    mu_sb[:, sl], hp[:, :K_OUT], b_head[:, :K_OUT], OP.add)
                nc.vector.tensor_tensor(
                    zs_sb[:, sl], hp[:, K_OUT:2 * K_OUT],
                    b_head[:, K_OUT:2 * K_OUT], OP.add)

            # ---------- main loop (encoder then decoder, unified body) ----------
            x_pipeline(0)
            x_pipeline(1)

            for step in range(t_enc + t_dec):
                enc = step < t_enc
                tau = step - t_enc
                nx = 1 if enc else HK  # x-input k-chunks of layer 0

                if enc and step + 2 < t_enc:
                    x_pipeline(step + 2)
                if not enc and tau == 0:
                    # decoder weights/biases: slots free as encoder tiles die
                    w["d0"] = load_w("d0", 2 * HK)
                    w["d1"] = load_w("d1", 2 * HK)
                    load_bias("d0")
                    load_bias("d1")
                wl0, wl1 = (w["e0"], w["e1"]) if enc else (w["d0"], w["d1"])
                bl0, bl1 = (bias["e0"], bias["e1"]) if enc else (bias["d0"], bias["d1"])

                # 1. L0 h-half
                psums0 = emit_mms([None] * NB, wl0, [hT[0][:, k, :] for k in range(HK)],
                                  list(range(nx, nx + HK)), start=True, stop=False)
                # 2. transpose h1(prev)
                if step > 0:
                    emit_tpose(1)
                # 3. heads for previous decoder output
                if not enc and tau > 0:
                    emit_heads(tau - 1)
                # 4. L0 x-half
                x_lhsT = [xT_ring[step % XRING]] if enc else [hT[1][:, k, :] for k in range(HK)]
                emit_mms(psums0, wl0, x_lhsT, list(range(nx)), start=False, stop=True)
                # 5. L0 elementwise
                emit_el(0, psums0, bl0)
                # 6. L1 h-half
                psums1 = emit_mms([None] * NB, wl1, [hT[1][:, k, :] for k in range(HK)],
                                  list(range(HK, 2 * HK)), start=True, stop=False)
                # 7. transpose h0 -> hT0
                emit_tpose(0)
                # 8. L1 x-half
                emit_mms(psums1, wl1, [hT[0][:, k, :] for k in range(HK)],
                         list(range(HK)), start=False, stop=True)
                # 9. L1 elementwise
                emit_el(1, psums1, bl1)

            # final decoder output
            emit_tpose(1)
            emit_heads(t_dec - 1)

            # sigma = softplus(2z)/2 = ln(1 + exp(2z))/2 (no softplus table
            # on ACT; exp/ln share one table set, loaded once here)
            et = tmps.tile([BC, t_dec * K_OUT], f32, tag="e")
            nc.scalar.activation(et, zs_sb, AF.Exp, scale=2.0)
            nc.scalar.activation(sg_sb, et, AF.Ln, bias=1.0)
            nc.vector.tensor_scalar_mul(sg_sb, sg_sb, 0.5)
            nc.sync.dma_start(
                mu_d[:, :, :], mu_sb.rearrange("b (t k) -> b t k", k=K_OUT))
            nc.sync.dma_start(
                sg_d[:, :, :], sg_sb.rearrange("b (t k) -> b t k", k=K_OUT))

    nc.finalize()
    return nc


def prep_weights(inp, t_enc=T):
    """Host-side weight layout prep. Returns the shared (non-x) input map."""
    def wcat(wih, whh, nk_x):
        # K-space rows: [x-input dims (padded to nk_x*128), h dims]
        din = wih.shape[1]
        xpart = np.zeros((nk_x * 128, G), np.float32)
        xpart[:din] = wih.T
        return np.concatenate([xpart, whh.T.astype(np.float32)], axis=0) \
            .reshape(nk_x + HK, 128, G)

    m = {}
    m["w_e0"] = wcat(inp["enc_Wih0"], inp["enc_Whh0"], 1)
    m["w_e1"] = wcat(inp["enc_Wih1"], inp["enc_Whh1"], HK)
    m["w_d0"] = wcat(inp["dec_Wih0"], inp["dec_Whh0"], HK)
    m["w_d1"] = wcat(inp["dec_Wih1"], inp["dec_Whh1"], HK)
    for nm, pre in (("e0", "enc_"), ("e1", "enc_"), ("d0", "dec_"), ("d1", "dec_")):
        i = nm[1]
        bsum = (inp[f"{pre}bih{i}"] + inp[f"{pre}bhh{i}"]).astype(np.float32)
        m[f"b_{nm}"] = np.ascontiguousarray(np.broadcast_to(bsum, (BC, G)))
    wh = np.concatenate([inp["W1"].T, inp["W2"].T], axis=1).astype(np.float32)  # [H, 16]
    m["w_head"] = np.ascontiguousarray(wh.reshape(HK, 128, 2 * K_OUT))
    bh = np.concatenate([inp["b1"], inp["b2"]]).astype(np.float32)
    m["b_head"] = np.ascontiguousarray(np.broadcast_to(bh, (BC, 2 * K_OUT)))
    m["ident"] = np.eye(128, dtype=np.float32)
    return m


_NC_CACHE = {}


def get_nc(t_enc=T, t_dec=TAU):
    key = (t_enc, t_dec)
    if key not in _NC_CACHE:
        _NC_CACHE[key] = build_nc(t_enc, t_dec)
    return _NC_CACHE[key]


def kernel(**inputs):
    inputs = {k: np.asarray(v) for k, v in inputs.items()}
    nc = get_nc()
    base = prep_weights(inputs)
    x = inputs["x"].astype(np.float32)
    in_maps = [dict(base, x=np.ascontiguousarray(x[i * BC:(i + 1) * BC]))
               for i in range(NCORES)]
    res = run_bass_kernel_spmd(nc, in_maps, core_ids=list(range(NCORES)))
    mu = np.concatenate([r["mu"] for r in res.results], axis=0)
    sigma = np.concatenate([r["sigma"] for r in res.results], axis=0)
    return mu, sigma



# revision 6
# speedup vs baseline: 1.3660x; 1.3660x over previous
"""DeepAR (2-layer LSTM encoder/decoder + gaussian heads) on 8 Trainium2 cores.

Data-parallel over batch B=1024 -> 128 rows/core. v2 design:

  - All LSTM matmuls in fp16 (1 col/cycle on PE, same as fp32r, but enables
    XBAR DMA transposes + FWL). fp32 PSUM accumulate. Numerically validated:
    max rel err ~7.5e-3 vs fp64 (tolerance 2e-2).
  - h transposes run on the (otherwise idle) DMA engines via the XBAR
    transpose, not the PE. No PE transpose / DVE copyback anywhere.
  - Elementwise uses a tanh-only formulation to halve ACT work:
      sig(x) = (tanh(x/2)+1)/2, states kept doubled (C2=2c, H2=2h) with the
      1/2 factors folded into all h-consuming weights host-side. Per cell:
      1 tanh over all four gates [128,2048] + 1 tanh(C2/2), plus 4 fused
      scalar_tensor_tensor ops split across DVE and GpSimd(Pool).
  - L0-encoder bias enters through two extra K-rows of the x-chunk matmul
    (ones rows x (bias_hi + bias_lo) fp16 pair = fp32-accurate bias).
    L1/decoder biases via one DVE STT per PSUM bank.
  - Heads need ~fp32 weights: W1/W2 split into fp16 hi+lo pairs, two
    accumulating matmuls each (input h stays fp16 - validated).
"""

import numpy as np
import ml_dtypes

import concourse.bass as bass
import concourse.mybir as mybir
import concourse.tile as tile
from concourse.bacc import Bacc
from concourse.bass_utils import run_bass_kernel_spmd

f32 = mybir.dt.float32
f16 = mybir.dt.float16
AF = mybir.ActivationFunctionType
OP = mybir.AluOpType

B, T, D, H, K_OUT, TAU = 1024, 168, 32, 512, 8, 24
NCORES = 8
BC = B // NCORES          # 128 batch rows per core
G = 4 * H                 # 2048 gate width
NB = G // 512             # 4 psum banks per layer-step
HK = H // 128             # 4 hT chunks
KX = D + 2                # x rows + 2 bias rows (hi+lo)

# gate slices (PyTorch order i, f, g, o)
SI, SF, SG, SO = (slice(k * H, (k + 1) * H) for k in range(4))


def build_nc(t_enc=T, t_dec=TAU):
    nc = Bacc()

    xt_d = nc.dram_tensor("xt", [KX, t_enc * BC], f16, kind="ExternalInput")
    w_d = {}
    # per-layer K-space chunk order: [own-h (4) | input-h (4)] ; e0 has the
    # x chunk separate (KX rows).
    w_d["e0h"] = nc.dram_tensor("w_e0h", [HK, 128, G], f16, kind="ExternalInput")
    w_d["e0x"] = nc.dram_tensor("w_e0x", [KX, G], f16, kind="ExternalInput")
    for nm in ("e1", "d0", "d1"):
        w_d[nm] = nc.dram_tensor(f"w_{nm}", [2 * HK, 128, G], f16, kind="ExternalInput")
    b_d = {nm: nc.dram_tensor(f"b_{nm}", [BC, G], f32, kind="ExternalInput")
           for nm in ("e1", "d0", "d1")}
    wh_d = nc.dram_tensor("w_head", [2, HK, 128, 2 * K_OUT], f16, kind="ExternalInput")
    bh_d = nc.dram_tensor("b_head", [BC, 2 * K_OUT], f32, kind="ExternalInput")
    mu_d = nc.dram_tensor("mu", [BC, t_dec, K_OUT], f32, kind="ExternalOutput")
    sg_d = nc.dram_tensor("sigma", [BC, t_dec, K_OUT], f32, kind="ExternalOutput")

    with tile.TileContext(nc) as tc:
        with (
            tc.tile_pool(name="consts", bufs=1) as consts,
            tc.tile_pool(name="wpool", bufs=21) as wpool,
            tc.tile_pool(name="bpool", bufs=2) as bpool,
            tc.tile_pool(name="tmps", bufs=8) as tmps,
            tc.tile_pool(name="th32", bufs=2) as thp,
            tc.tile_pool(name="gps", bufs=2, space="PSUM") as gps,
        ):
            # ---------- startup loads ----------
            xt_sb = consts.tile([KX, t_enc * BC], f16, tag="xt")
            nc.sync.dma_start(xt_sb, xt_d[:, :])

            # partition dim must lead: store as [128, 2, HK, 2K]
            w_head = consts.tile([128, 2, HK, 2 * K_OUT], f16, tag="w_head")
            nc.sync.dma_start(w_head, wh_d[:, :, :, :].rearrange("h k p n -> p h k n"))
            b_head = consts.tile([BC, 2 * K_OUT], f32, tag="b_head")
            nc.sync.dma_start(b_head, bh_d[:, :])

            def load_w(nm, nk):
                chunks = []
                for k in range(nk):
                    wt = wpool.tile([128, G], f16, tag="w")
                    nc.sync.dma_start(wt, w_d[nm][k, :, :])
                    chunks.append(wt)
                return chunks

            w = {"e0h": load_w("e0h", HK), "e1": load_w("e1", 2 * HK)}
            w_e0x = consts.tile([KX, G], f16, tag="w_e0x")
            nc.sync.dma_start(w_e0x, w_d["e0x"][:, :])

            bias = {}

            def load_bias(nm):
                bias[nm] = bpool.tile([BC, G], f32, tag="b", name=f"b_{nm}")
                nc.sync.dma_start(bias[nm], b_d[nm][:, :])

            load_bias("e1")

            # ---------- persistent state ----------
            hT = {}
            c2 = {}
            h_tmp = {}
            for l in (0, 1):
                hT[l] = consts.tile([128, HK, BC], f16, tag=f"hT{l}", name=f"hT{l}")
                nc.vector.memset(hT[l], 0.0)
                c2[l] = consts.tile([BC, H], f32, tag=f"c2_{l}", name=f"c2_{l}")
                nc.vector.memset(c2[l], 0.0)
                h_tmp[l] = consts.tile([BC, H], f16, tag=f"h_{l}", name=f"h_{l}")

            mu_sb = consts.tile([BC, t_dec * K_OUT], f32, tag="mu_sb")
            zs_sb = consts.tile([BC, t_dec * K_OUT], f32, tag="zs_sb")
            sg_sb = consts.tile([BC, t_dec * K_OUT], f32, tag="sg_sb")

            # ---------- helpers ----------
            def emit_bank(psum, n, pairs, start, stop):
                """pairs: list of (lhsT, w_chunk); emit the bank-n matmuls."""
                ns = slice(n * 512, (n + 1) * 512)
                for j, (lh, wt) in enumerate(pairs):
                    nc.tensor.matmul(
                        psum[:, ns], lh, wt[:, ns],
                        start=start and j == 0,
                        stop=stop and j == len(pairs) - 1)

            def emit_el(l, psum, b_t):
                """Tanh-only LSTM cell. psum holds W.x-contributions for all
                four gates; bias b_t (or None if folded into the matmul)."""
                th = thp.tile([BC, G], f32, tag="th")
                for n in range(NB):
                    ns = slice(n * 512, (n + 1) * 512)
                    # bank 2 is the g-gate: needs tanh(g); the sigmoid banks
                    # (i, f, o) reconstruct via tanh(x/2)
                    sc = 1.0 if n == 2 else 0.5
                    if b_t is not None:
                        z = thp.tile([BC, 512], f32, tag="z", bufs=8)
                        nc.vector.scalar_tensor_tensor(
                            z, psum[:, ns], 1.0, b_t[:, ns], OP.mult, OP.add)
                        nc.scalar.activation(th[:, ns], z, AF.Tanh, scale=sc)
                    else:
                        nc.scalar.activation(th[:, ns], psum[:, ns], AF.Tanh,
                                             scale=sc)
                # a = (th_f + 1) * C2  (emitted first: th_f lands early)
                at = tmps.tile([BC, H], f32, tag="e")
                nc.vector.scalar_tensor_tensor(at, th[:, SF], 1.0, c2[l],
                                               OP.add, OP.mult)
                # b = (th_i + 1) * th_g
                bt = tmps.tile([BC, H], f32, tag="e")
                nc.vector.scalar_tensor_tensor(bt, th[:, SI], 1.0, th[:, SG],
                                               OP.add, OP.mult)
                # C2 = a*0.5 + b
                nc.vector.scalar_tensor_tensor(c2[l], at, 0.5, bt,
                                               OP.mult, OP.add)
                # tc = tanh(C2/2)                (ACT)
                tc_ = tmps.tile([BC, H], f32, tag="e")
                nc.scalar.activation(tc_, c2[l], AF.Tanh, scale=0.5)
                # H2 = (th_o + 1) * tc -> fp16   (DVE)
                nc.vector.scalar_tensor_tensor(h_tmp[l], th[:, SO], 1.0, tc_,
                                               OP.add, OP.mult)
                # hT via XBAR DMA transpose (4 chunks)
                for k in range(HK):
                    nc.sync.dma_start(hT[l][:, k, :],
                                      h_tmp[l][:, k * 128:(k + 1) * 128],
                                      transpose=True)

            def emit_heads(ti):
                """mu/sigma for decoder output ti from hT[1]; hi+lo weights."""
                hp = gps.tile([BC, G], f32, tag="g")
                mms = [(hT[1][:, k, :], w_head[:, hl, k, :])
                       for hl in (0, 1) for k in range(HK)]
                for j, (lh, wt) in enumerate(mms):
                    nc.tensor.matmul(hp[:, :2 * K_OUT], lh, wt,
                                     start=(j == 0), stop=(j == len(mms) - 1))
                sl = slice(ti * K_OUT, (ti + 1) * K_OUT)
                nc.vector.tensor_tensor(
                    mu_sb[:, sl], hp[:, :K_OUT], b_head[:, :K_OUT], OP.add)
                nc.vector.tensor_tensor(
                    zs_sb[:, sl], hp[:, K_OUT:2 * K_OUT],
                    b_head[:, K_OUT:2 * K_OUT], OP.add)

            # ---------- main loop ----------
            for step in range(t_enc + t_dec):
                enc = step < t_enc
                tau = step - t_enc

                if not enc and tau == 0:
                    w["d0"] = load_w("d0", 2 * HK)
                    w["d1"] = load_w("d1", 2 * HK)
                    load_bias("d0")
                    load_bias("d1")

                # --- layer 0 ---
                psum0 = gps.tile([BC, G], f32, tag="g")
                if enc:
                    # all deps old -> bank-complete order (bank0 stops early,
                    # elementwise starts while later banks stream)
                    xs = slice(step * BC, (step + 1) * BC)
                    pairs0 = ([(hT[0][:, k, :], w["e0h"][k]) for k in range(HK)]
                              + [(xt_sb[:, xs], w_e0x)])
                    for n in range(NB):
                        emit_bank(psum0, n, pairs0, start=True, stop=True)
                else:
                    # own-h pass first (hT0 is older than hT1 from prev step)
                    wd0 = w["d0"]
                    for n in range(NB):
                        emit_bank(psum0, n,
                                  [(hT[0][:, k, :], wd0[k]) for k in range(HK)],
                                  start=True, stop=False)
                    for n in range(NB):
                        emit_bank(psum0, n,
                                  [(hT[1][:, k, :], wd0[HK + k]) for k in range(HK)],
                                  start=False, stop=True)
                if not enc and tau > 0:
                    emit_heads(tau - 1)
                emit_el(0, psum0, None if enc else bias["d0"])

                # --- layer 1: own-h pass first (old dep) keeps the PE busy
                # while layer 0's elementwise + transposes produce hT0(t);
                # the input half (h0, fresh) streams second ---
                wl1 = w["e1"] if enc else w["d1"]
                bl1 = bias["e1"] if enc else bias["d1"]
                psum1 = gps.tile([BC, G], f32, tag="g")
                for n in range(NB):
                    emit_bank(psum1, n,
                              [(hT[1][:, k, :], wl1[k]) for k in range(HK)],
                              start=True, stop=False)
                for n in range(NB):
                    emit_bank(psum1, n,
                              [(hT[0][:, k, :], wl1[HK + k]) for k in range(HK)],
                              start=False, stop=True)
                emit_el(1, psum1, bl1)

            emit_heads(t_dec - 1)

            # sigma = softplus(2z)/2 = ln(1 + exp(2z))/2
            et = tmps.tile([BC, t_dec * K_OUT], f32, tag="fin")
            nc.scalar.activation(et, zs_sb, AF.Exp, scale=2.0)
            nc.scalar.activation(sg_sb, et, AF.Ln, bias=1.0)
            nc.vector.tensor_scalar_mul(sg_sb, sg_sb, 0.5)
            nc.sync.dma_start(
                mu_d[:, :, :], mu_sb.rearrange("b (t k) -> b t k", k=K_OUT))
            nc.sync.dma_start(
                sg_d[:, :, :], sg_sb.rearrange("b (t k) -> b t k", k=K_OUT))

    nc.finalize()
    return nc


def _f16_split(a):
    """Split fp32 array into (hi, lo) fp16 pair with hi+lo ~ fp32-accurate."""
    hi = a.astype(np.float16)
    lo = (a.astype(np.float64) - hi.astype(np.float64)).astype(np.float16)
    return hi, lo


def prep_weights(inp, t_enc=T):
    """Host-side layout prep. All h-consuming weights halved (H2=2h)."""
    m = {}

    def hchunks(w):  # [4H, 512] -> [HK, 128, G], halved
        return np.ascontiguousarray(
            (w.T.astype(np.float32) / 2.0).reshape(HK, 128, G).astype(np.float16))

    m["w_e0h"] = hchunks(inp["enc_Whh0"])
    # x chunk: rows 0..D-1 = Wih0.T (unscaled), rows D, D+1 = bias hi/lo
    e0x = np.zeros((KX, G), np.float16)
    e0x[:D] = inp["enc_Wih0"].T.astype(np.float16)
    b0 = (inp["enc_bih0"] + inp["enc_bhh0"]).astype(np.float32)
    e0x[D], e0x[D + 1] = _f16_split(b0)
    m["w_e0x"] = e0x

    for nm, pre in (("e1", "enc_"), ("d0", "dec_"), ("d1", "dec_")):
        i = nm[1]
        m[f"w_{nm}"] = np.concatenate(
            [hchunks(inp[f"{pre}Whh{i}"]), hchunks(inp[f"{pre}Wih{i}"])], axis=0)
        bsum = (inp[f"{pre}bih{i}"] + inp[f"{pre}bhh{i}"]).astype(np.float32)
        m[f"b_{nm}"] = np.ascontiguousarray(np.broadcast_to(bsum, (BC, G)))

    wh = np.concatenate([inp["W1"].T, inp["W2"].T], axis=1).astype(np.float32) / 2.0
    hi, lo = _f16_split(wh)  # [H, 2K]
    m["w_head"] = np.ascontiguousarray(
        np.stack([hi, lo]).reshape(2, HK, 128, 2 * K_OUT))
    bh = np.concatenate([inp["b1"], inp["b2"]]).astype(np.float32)
    m["b_head"] = np.ascontiguousarray(np.broadcast_to(bh, (BC, 2 * K_OUT)))
    return m


def make_xt(x_core, t_enc=T):
    """Per-core x -> [KX, t_enc*BC] fp16 with ones rows for the bias."""
    xt = np.zeros((KX, t_enc * BC), np.float16)
    xt[:D] = np.ascontiguousarray(
        x_core[:, :t_enc, :].transpose(2, 1, 0)).reshape(D, t_enc * BC)
    xt[D] = 1.0
    xt[D + 1] = 1.0
    return xt


_NC_CACHE = {}


def get_nc(t_enc=T, t_dec=TAU):
    key = (t_enc, t_dec)
    if key not in _NC_CACHE:
        _NC_CACHE[key] = build_nc(t_enc, t_dec)
    return _NC_CACHE[key]


def make_in_maps(inputs, t_enc=T):
    base = prep_weights(inputs, t_enc)
    x = inputs["x"].astype(np.float32)
    return [dict(base, xt=make_xt(x[i * BC:(i + 1) * BC], t_enc))
            for i in range(NCORES)]


def kernel(**inputs):
    inputs = {k: np.asarray(v) for k, v in inputs.items()}
    nc = get_nc()
    in_maps = make_in_maps(inputs)
    res = run_bass_kernel_spmd(nc, in_maps, core_ids=list(range(NCORES)))
    mu = np.concatenate([r["mu"] for r in res.results], axis=0)
    sigma = np.concatenate([r["sigma"] for r in res.results], axis=0)
    return mu, sigma
